# revision 45
# baseline (speedup 1.0000x reference)
"""MLA (multi-head latent attention) Trainium2 kernel.

Sharding: 8 cores = 2 (batch) x 4 (head groups of 4 heads).
Each core computes, for its batch b and heads [4g, 4g+4):
  latents kv_d/q_d (replicated within the batch group), per-head
  up-projections + RoPE, causal SDPA, and a partial o_proj
  out_core[o, q] = sum_{d in core's 512 head-dims} W_o[o, d] * y[d, q].
Host sums the 4 partials per batch (the all-reduce step of the hint,
performed at unshard time) and transposes to [S, H].

Key structure (v3):
  - Phase 1 streams xT once ([128,512] tiles, n-outer / k-inner) and
    computes kv_d, q_d and raw rope-k together in 6 PSUM banks, with
    packed per-k weight slices arriving just-in-time; k-RoPE runs
    per n-slice inside the loop.
  - SDPA runs per 512-query block at single 128-key-chunk granularity
    with a fine-grained diagonal (only the causal trapezoid is computed).
  - The softmax denominator is accumulated on the vector engine (bf16)
    across key chunks; a single ones-matmul per (head, q-block) reduces
    it across partitions.
  - PSUM->SBUF drains are split between the scalar and vector engines.

All matmuls run in bf16 with fp32 PSUM accumulation.
"""

import numpy as np
import ml_dtypes

import concourse.bass as bass
import concourse.mybir as mybir
import concourse.tile as tile
from concourse import bacc
from concourse._compat import get_trn_type
from concourse.bass_utils import run_bass_kernel_spmd

H = 2048
NH = 16
HD = 128           # head dim
RD = 64            # rotary dim
RH = 32            # rotary half
LAT = 256
B = 2
S = 2048
BASE = 10000.0
N_CORES = 8
HEADS_PER_CORE = 4
P = 128
NQB = S // 512     # 4 query blocks of 512
NKI = S // 128     # 16 key chunks of 128
SCALE = 1.0 / np.sqrt(float(HD))
EXP_BIAS = -4.0

BF16 = mybir.dt.bfloat16
F32 = mybir.dt.float32
_bf = ml_dtypes.bfloat16


def _mm(nc, out, lhsT, rhs, start, stop):
    nc.tensor.matmul(out, lhsT, rhs, start=start, stop=stop)


def build_program():
    nc = bacc.Bacc(
        get_trn_type() or "TRN2",
        target_bir_lowering=False,
        debug=False,
        num_devices=N_CORES,
    )

    xT = nc.declare_dram_parameter("xT", [H, S], BF16, isOutput=False)
    # packed phase-1 weights: per k-chunk [wrk(256) | wkvd(256) | wqd(256)]
    w_ph1 = nc.declare_dram_parameter("w_ph1", [P, 16, 768], BF16,
                                      isOutput=False)
    w_qc = nc.declare_dram_parameter("w_qc", [P, 2, 512], BF16, isOutput=False)
    w_kc = nc.declare_dram_parameter("w_kc", [P, 2, 256], BF16, isOutput=False)
    w_v = nc.declare_dram_parameter("w_v", [P, 2, 512], BF16, isOutput=False)
    w_o = nc.declare_dram_parameter("w_o", [P, 4, H], BF16, isOutput=False)
    cosA = nc.declare_dram_parameter("cosA", [P, S], BF16, isOutput=False)
    sinB = nc.declare_dram_parameter("sinB", [P, S], BF16, isOutput=False)
    masks = nc.declare_dram_parameter("masks", [P, P], BF16, isOutput=False)
    out = nc.declare_dram_parameter("out", [H, S], BF16, isOutput=True)

    Exp = mybir.ActivationFunctionType.Exp
    Mult = mybir.AluOpType.mult
    Add = mybir.AluOpType.add

    with tile.TileContext(nc) as tc:
        with (
            tc.tile_pool(name="main", bufs=1) as main,
        ):
            # -------- persistent small tensors --------
            cos_sb = main.tile([P, S], BF16, tag="cos", name="cos")
            sin_sb = main.tile([P, S], BF16, tag="sin", name="sin")
            mask_sb = main.tile([P, P], BF16, tag="mask", name="mask")
            ones_sb = main.tile([P, P], BF16, tag="ones", name="ones")
            nc.gpsimd.memset(ones_sb[:], 1.0)
            ebias_sb = main.tile([P, 1], F32, tag="ebias", name="ebias")
            nc.gpsimd.memset(ebias_sb[:], EXP_BIAS)
            wo_sb = main.tile([P, 4, H], BF16, tag="wo", name="wo")
            wqc_sb = main.tile([P, 2, 512], BF16, tag="wqc", name="wqc")
            wkc_sb = main.tile([P, 2, 256], BF16, tag="wkc", name="wkc")
            wv_sb = main.tile([P, 2, 512], BF16, tag="wv", name="wv")

            # -------- phase-1 outputs (latents + raw rope-k), bf16 --------
            kvd_sb = [main.tile([P, S], BF16, tag=f"kvd{m}", name=f"kvd{m}")
                      for m in range(2)]
            qd_sb = [main.tile([P, S], BF16, tag=f"qd{m}", name=f"qd{m}")
                     for m in range(2)]
            kraw = [main.tile([P, S], BF16, tag=f"kraw{p}", name=f"kraw{p}")
                    for p in range(2)]
            swpk = [main.tile([P, S], BF16, tag=f"swpk{p}", name=f"swpk{p}")
                    for p in range(2)]

            # phase-1 weights (packed)
            wph1_sb = main.tile([P, 16, 768], BF16, tag="wph1", name="wph1")

            # -------- DMA staging: critical-first --------
            # per-k packed weight slices, alternating between the scalar
            # and gpsimd queues (small transfers on two queues keep the
            # descriptor round-robin fair to the sync-queue x stream).
            for k in range(16):
                eng = nc.scalar if k % 2 == 0 else nc.gpsimd
                eng.dma_start(wph1_sb[:, k:k + 1, :], w_ph1[:, k:k + 1, :])
            # Phase-2/SDPA tensors are gated on n=0 completing (see n-loop).

            # -------- per-head q/k tiles (dims on partitions), v, y --------
            # even head h: rows [0:64] content, [64:128] rope
            # odd  head h: rows [0:64] rope,    [64:128] content
            qT = [main.tile([P, S], BF16, tag=f"qT{h}", name=f"qT{h}")
                  for h in range(4)]
            kT = [main.tile([P, S], BF16, tag=f"kT{h}", name=f"kT{h}")
                  for h in range(4)]
            v_sb = main.tile([P, NKI, 512], BF16, tag="v", name="v")
            y_sb = [main.tile([P, S], BF16, tag=f"y{h}", name=f"y{h}")
                    for h in range(4)]

            # -------- phase 1 (merged): stream xT once, n-outer/k-inner.
            # 6 PSUM banks per n-chunk: kr_g0, kr_g1, kvd0, kvd1, qd0, qd1.
            with tc.tile_pool(name="ps1", bufs=1, space="PSUM") as ps1:
                for n in range(4):
                    ns = slice(n * 512, (n + 1) * 512)
                    pb = [ps1.tile([P, 512], F32, tag=f"b{i}", name=f"p1_{i}")
                          for i in range(6)]
                    for k in range(16):
                        xtk = main.tile([P, 512], BF16, tag="xtk", name="xtk",
                                       bufs=10)
                        nc.sync.dma_start(
                            xtk[:], xT[k * 128:(k + 1) * 128, ns])
                        for i in range(6):
                            _mm(nc, pb[i][:],
                                wph1_sb[:, k, i * 128:(i + 1) * 128],
                                xtk[:], k == 0, k == 15)
                    # drain in next-iteration need order, split across engines
                    nc.vector.tensor_copy(kraw[0][:, ns], pb[0][:])
                    nc.scalar.copy(kraw[1][:, ns], pb[1][:])
                    nc.vector.tensor_copy(kvd_sb[0][:, ns], pb[2][:])
                    nc.scalar.copy(kvd_sb[1][:, ns], pb[3][:])
                    nc.vector.tensor_copy(qd_sb[0][:, ns], pb[4][:])
                    nc.scalar.copy(qd_sb[1][:, ns], pb[5][:])
                    if n == 0:
                        # phase-2/SDPA tensors: the tiny gpsimd copy reads an
                        # n=0 product, so the gpsimd queue actually WAITS for
                        # n=0 before issuing these loads (keeps HBM bandwidth
                        # for the phase-1 x/weight stream).
                        gate = main.tile([P, 8], BF16, tag="gate", name="gate",
                                        bufs=2)
                        nc.gpsimd.tensor_copy(gate[:], kvd_sb[0][:, 504:512])
                        nc.gpsimd.dma_start(cos_sb[:], cosA[:, :])
                        nc.gpsimd.dma_start(sin_sb[:], sinB[:, :])
                        nc.gpsimd.dma_start(mask_sb[:], masks[:, :])
                        nc.gpsimd.dma_start(wqc_sb[:], w_qc[:, :, :])
                        nc.gpsimd.dma_start(wkc_sb[:], w_kc[:, :, :])
                        nc.gpsimd.dma_start(wv_sb[:], w_v[:, :, :])
                    if n == 1:
                        gate = main.tile([P, 8], BF16, tag="gate", name="gate",
                                        bufs=2)
                        nc.gpsimd.tensor_copy(gate[:],
                                              kvd_sb[0][:, 1016:1024])
                        nc.gpsimd.dma_start(wo_sb[:], w_o[:, :, :])
                    if n > 0:
                        # rope on k for the PREVIOUS n-slice (cos/sin tables
                        # only arrive after the n=0 gate fires)
                        _rope_k(nc, n - 1, kraw, swpk, kT, cos_sb, sin_sb)
                _rope_k(nc, 3, kraw, swpk, kT, cos_sb, sin_sb)

            # -------- phase 2 + SDPA + o_proj (single PSUM epoch) --------
            # Phase-2 up-projections ride the "opj" PSUM ring per n-slice,
            # so SDPA q-block n never waits at a pool boundary for later
            # phase-2 work; ph2_chunk(n+1) is issued between SDPA heads and
            # o_proj to fill the softmax-tail bubble.
            with tc.tile_pool(name="ps3", bufs=1, space="PSUM") as ps3:
                swpq = main.tile([P, S], BF16, tag="swpq", name="swpq")

                def ph2_chunk(n):
                    ns = slice(n * 512, (n + 1) * 512)
                    # k content per pair: psum rows [0:64]=h0, [64:128]=h1
                    for p in range(2):
                        h0, h1 = 2 * p, 2 * p + 1
                        pt = ps3.tile([P, 512], F32, tag="opj", name="kcont",
                                      bufs=2)
                        for kc in range(2):
                            _mm(nc, pt[:],
                                wkc_sb[:, kc, p * 128:(p + 1) * 128],
                                kvd_sb[kc][:, ns], kc == 0, kc == 1)
                        nc.scalar.copy(kT[h0][0:64, ns], pt[0:64, :])
                        nc.scalar.copy(kT[h1][64:128, ns], pt[64:128, :])
                    # q combined up-proj + rope per head
                    for h in range(4):
                        r = 64 if h % 2 == 0 else 0
                        pt = ps3.tile([P, 512], F32, tag="opj", name="qcomb",
                                      bufs=2)
                        for kc in range(2):
                            _mm(nc, pt[:],
                                wqc_sb[:, kc, h * 128:(h + 1) * 128],
                                qd_sb[kc][:, ns], kc == 0, kc == 1)
                        nc.scalar.copy(qT[h][:, ns], pt[:])
                        nc.sync.dma_start(swpq[r:r + 32, ns],
                                          qT[h][r + 32:r + 64, ns])
                        nc.sync.dma_start(swpq[r + 32:r + 64, ns],
                                          qT[h][r:r + 32, ns])
                        nc.vector.tensor_mul(
                            qT[h][r:r + 64, ns], qT[h][r:r + 64, ns],
                            cos_sb[r:r + 64, ns]
                        )
                        nc.vector.tensor_mul(
                            swpq[r:r + 64, ns], swpq[r:r + 64, ns],
                            sin_sb[r:r + 64, ns]
                        )
                        nc.vector.tensor_add(
                            qT[h][r:r + 64, ns], qT[h][r:r + 64, ns],
                            swpq[r:r + 64, ns]
                        )
                    # v chunks for this n-slice (all 4 heads x 128 dims)
                    for s16 in range(4 * n, 4 * n + 4):
                        pt = ps3.tile([P, 512], F32, tag="opj", name="vps",
                                      bufs=2)
                        for kc in range(2):
                            _mm(nc, pt[:],
                                kvd_sb[kc][:, s16 * 128:(s16 + 1) * 128],
                                wv_sb[:, kc, 0:512], kc == 0, kc == 1)
                        nc.vector.tensor_copy(v_sb[:, s16, :], pt[:])

                def emit_oproj(qbp, ocs):
                    """Partial o_proj for q-block ``qbp`` (a few oc chunks)."""
                    qsp = slice(qbp * 512, (qbp + 1) * 512)
                    for oc in ocs:
                        opt_ = ps3.tile([P, 512], F32, tag="opj", name="opj",
                                        bufs=2)
                        for hk in range(4):
                            _mm(nc, opt_[:],
                                wo_sb[:, hk, oc * 128:(oc + 1) * 128],
                                y_sb[hk][:, qsp], hk == 0, hk == 3)
                        osb = main.tile([P, 512], BF16, tag="osb", name="osb",
                                        bufs=4)
                        if oc % 2 == 0:
                            nc.scalar.copy(osb[:], opt_[:])
                        else:
                            nc.vector.tensor_copy(osb[:], opt_[:])
                        nc.sync.dma_start(out[oc * 128:(oc + 1) * 128, qsp],
                                          osb[:])

                ph2_chunk(0)
                for qb in range(NQB):
                    qs = slice(qb * 512, (qb + 1) * 512)
                    nod = 4 * qb          # off-diagonal chunks (pairs)
                    for h in range(4):
                        hv = h * 128
                        yps = ps3.tile([P, 512], F32, tag="ypv", name="ypv",
                                       bufs=2)
                        # bf16 denominator accumulator; the first exp writes
                        # it directly (doubles as that pair's prb)
                        tacc = main.tile([P, 1024], BF16, tag="tacc",
                                        name="tacc", bufs=2)
                        # off-diagonal chunk pairs: full width
                        for g in range(nod // 2):
                            scps = ps3.tile([P, 1024], F32, tag="sc",
                                            name="sc", bufs=2)
                            for j in range(2):
                                ki = 2 * g + j
                                _mm(nc, scps[:, j * 512:(j + 1) * 512],
                                    kT[h][:, ki * 128:(ki + 1) * 128],
                                    qT[h][:, qs], True, True)
                            if g == 0:
                                prb = tacc
                            else:
                                prb = main.tile([P, 1024], BF16, tag="prb",
                                               name="prb", bufs=4)
                            nc.scalar.activation(
                                prb[:], scps[:], Exp,
                                bias=ebias_sb[:], scale=SCALE
                            )
                            if g > 0:
                                nc.vector.tensor_add(tacc[:], tacc[:], prb[:])
                            for j in range(2):
                                ki = 2 * g + j
                                _mm(nc, yps[:],
                                    v_sb[:, ki, hv:hv + 128],
                                    prb[:, j * 512:(j + 1) * 512],
                                    ki == 0, False)
                        # diagonal chunks: shrinking width + triangular mask
                        for l in range(4):
                            ki = nod + l
                            c0 = 128 * l
                            cs = slice(c0, 512)
                            scps = ps3.tile([P, 1024], F32, tag="sc",
                                            name="sc", bufs=2)
                            _mm(nc, scps[:, cs],
                                kT[h][:, ki * 128:(ki + 1) * 128],
                                qT[h][:, qb * 512 + c0:(qb + 1) * 512],
                                True, True)
                            if qb == 0 and l == 0:
                                prb = tacc
                            else:
                                prb = main.tile([P, 1024], BF16, tag="prb",
                                               name="prb", bufs=4)
                            nc.scalar.activation(
                                prb[:, cs], scps[:, cs], Exp,
                                bias=ebias_sb[:], scale=SCALE
                            )
                            nc.gpsimd.tensor_mul(
                                prb[:, c0:c0 + 128],
                                prb[:, c0:c0 + 128], mask_sb[:]
                            )
                            if not (qb == 0 and l == 0):
                                nc.vector.tensor_add(
                                    tacc[:, cs], tacc[:, cs], prb[:, cs])
                            # AV: start=True clears has_written for the WHOLE
                            # bank, so it appears exactly once (ki==0, which
                            # is always full width).
                            _mm(nc, yps[:, cs],
                                v_sb[:, ki, hv:hv + 128],
                                prb[:, cs], ki == 0, l == 3)
                        # reduce denominator across partitions: accumulate
                        # both tacc halves into one PSUM region (no DVE fold
                        # on the critical path)
                        rsps = ps3.tile([P, 1024], F32, tag="sc", name="rs",
                                        bufs=2)
                        _mm(nc, rsps[:, 0:512], ones_sb[:], tacc[:, 0:512],
                            True, qb == 0)
                        if qb > 0:
                            _mm(nc, rsps[:, 0:512], ones_sb[:],
                                tacc[:, 512:1024], False, True)
                        rcp = main.tile([P, 512], F32, tag="rcp", name="rcp",
                                       bufs=2)
                        nc.vector.reciprocal_approx_fast(rcp[:], rsps[:, 0:512])
                        nc.vector.tensor_mul(y_sb[h][:, qs], yps[:], rcp[:])
                        if qb > 0:
                            # interleave the previous q-block's o_proj: ready
                            # tensor work that absorbs exp-paced score stalls
                            emit_oproj(qb - 1, range(4 * h, 4 * h + 4))
                    if qb < 3:
                        # next n-slice's up-projections fill the tail bubble
                        ph2_chunk(qb + 1)
                emit_oproj(3, range(16))

    nc.compile()
    return nc


def _rope_k(nc, n, kraw, swpk, kT, cos_sb, sin_sb):
    """Apply RoPE to the raw rope-k slices for n-chunk ``n``.

    kraw[p] rows [0:64] = h1 rope dims, [64:128] = h0 rope dims.
    swp DMAs ride the sync queue; the muls/adds ride the vector engine
    (both are otherwise idle mid-phase-1).
    """
    ns = slice(n * 512, (n + 1) * 512)
    for p in range(2):
        h0, h1 = 2 * p, 2 * p + 1
        sw = swpk[p]
        nc.sync.dma_start(sw[0:32, ns], kraw[p][32:64, ns])
        nc.sync.dma_start(sw[32:64, ns], kraw[p][0:32, ns])
        nc.sync.dma_start(sw[64:96, ns], kraw[p][96:128, ns])
        nc.sync.dma_start(sw[96:128, ns], kraw[p][64:96, ns])
        nc.vector.tensor_mul(sw[:, ns], sw[:, ns], sin_sb[:, ns])
        nc.vector.tensor_mul(
            kT[h1][0:64, ns], kraw[p][0:64, ns], cos_sb[0:64, ns]
        )
        nc.vector.tensor_add(
            kT[h1][0:64, ns], kT[h1][0:64, ns], sw[0:64, ns]
        )
        nc.vector.tensor_mul(
            kT[h0][64:128, ns], kraw[p][64:128, ns], cos_sb[64:128, ns]
        )
        nc.vector.tensor_add(
            kT[h0][64:128, ns], kT[h0][64:128, ns], sw[64:128, ns]
        )


_NC = None


def _get_nc():
    global _NC
    if _NC is None:
        _NC = build_program()
    return _NC


def _rope_tables():
    """cosA/sinB [128, S]: 32-row frequency pattern tiled 4x.
    sinB sign: rows [0:32] of each 64-block -> -sin, rows [32:64] -> +sin."""
    inv_freq = 1.0 / (BASE ** (np.arange(0, RD, 2, dtype=np.float32) / RD))  # [32]
    pos = np.arange(S, dtype=np.float32)
    ang = inv_freq[:, None] * pos[None, :]              # [32, S]
    cos1, sin1 = np.cos(ang), np.sin(ang)
    cosA = np.tile(cos1, (4, 1))                        # [128, S]
    sinB = np.concatenate([-sin1, sin1, -sin1, sin1], axis=0)
    return cosA.astype(_bf), sinB.astype(_bf)


def _mask_tiles():
    """mask[k, q] = 1.0 if q >= k else 0 (bf16, [128,128] triangular)."""
    k = np.arange(P)[:, None]
    q = np.arange(P)[None, :]
    return (q >= k).astype(np.float32).astype(_bf)


def _prep_core_inputs(c, x, W_kv_d, W_q_d, W_k_u, W_q_u, W_v_u, W_rope_k, W_rope_q,
                      W_o, cosA, sinB, masks):
    b = c // 4
    hg = c % 4
    heads = [4 * hg + j for j in range(HEADS_PER_CORE)]

    def tile_pmaj(w):
        # [ko*128, m] -> [128, ko, m] partition-major for contiguous DMA
        ko = w.shape[0] // P
        return np.ascontiguousarray(
            w.reshape(ko, P, w.shape[1]).transpose(1, 0, 2))

    xT = np.ascontiguousarray(x[b].T).astype(_bf)                  # [H, S]
    w_kvd = tile_pmaj(np.ascontiguousarray(W_kv_d.T).astype(_bf))
    w_qd = tile_pmaj(np.ascontiguousarray(W_q_d.T).astype(_bf))

    # w_rk: per pair, rows [h1 rope dims | h0 rope dims], then transpose
    blocks = []
    for p in range(2):
        g0, g1 = heads[2 * p], heads[2 * p + 1]
        blocks.append(W_rope_k[g1 * RD:(g1 + 1) * RD, :])
        blocks.append(W_rope_k[g0 * RD:(g0 + 1) * RD, :])
    w_rk = tile_pmaj(np.ascontiguousarray(np.concatenate(blocks, axis=0).T).astype(_bf))

    # packed phase-1 weights: per k-chunk [wrk | wkvd | wqd]
    w_ph1 = np.ascontiguousarray(
        np.concatenate([w_rk, w_kvd, w_qd], axis=2))

    # w_qc: per local head 128 cols: even -> [content|rope], odd -> [rope|content]
    cols = []
    for j, g in enumerate(heads):
        c_blk = W_q_u[g * RD:(g + 1) * RD, :].T       # [LAT, 64]
        r_blk = W_rope_q[g * RD:(g + 1) * RD, :].T    # [LAT, 64]
        cols.extend([c_blk, r_blk] if j % 2 == 0 else [r_blk, c_blk])
    w_qc = tile_pmaj(np.ascontiguousarray(np.concatenate(cols, axis=1)).astype(_bf))

    # w_kc: per pair 128 cols: [h0 content | h1 content]
    cols = []
    for p in range(2):
        g0, g1 = heads[2 * p], heads[2 * p + 1]
        cols.append(W_k_u[g0 * RD:(g0 + 1) * RD, :].T)
        cols.append(W_k_u[g1 * RD:(g1 + 1) * RD, :].T)
    w_kc = tile_pmaj(np.ascontiguousarray(np.concatenate(cols, axis=1)).astype(_bf))

    # w_v: per head 128 cols, heads in order (512 total)
    cols = [W_v_u[g * HD:(g + 1) * HD, :].T for g in heads]
    w_v = tile_pmaj(np.ascontiguousarray(np.concatenate(cols, axis=1)).astype(_bf))

    d0 = heads[0] * HD
    w_o = tile_pmaj(np.ascontiguousarray(W_o[:, d0:d0 + 512].T).astype(_bf))

    return {
        "xT": xT, "w_ph1": w_ph1, "w_qc": w_qc,
        "w_kc": w_kc, "w_v": w_v, "w_o": w_o, "cosA": cosA, "sinB": sinB,
        "masks": masks,
    }


def make_in_maps(inputs):
    x = np.asarray(inputs["hidden_states"], dtype=np.float32)
    ws = {k: np.asarray(inputs[k], dtype=np.float32)
          for k in ("W_kv_d", "W_q_d", "W_k_u", "W_q_u", "W_v_u", "W_rope_k",
                    "W_rope_q", "W_o")}
    cosA, sinB = _rope_tables()
    masks = _mask_tiles()
    return [
        _prep_core_inputs(c, x, ws["W_kv_d"], ws["W_q_d"], ws["W_k_u"],
                          ws["W_q_u"], ws["W_v_u"], ws["W_rope_k"],
                          ws["W_rope_q"], ws["W_o"], cosA, sinB, masks)
        for c in range(N_CORES)
    ]


def assemble(results):
    """results: list of 8 dicts with 'out' [H, S] partials (transposed)."""
    full = np.empty((B, S, H), dtype=np.float32)
    for b in range(B):
        acc = results[4 * b]["out"].astype(np.float32)
        for g in range(1, 4):
            acc = acc + results[4 * b + g]["out"]
        full[b] = acc.T
    return full


def kernel(**inputs):
    nc = _get_nc()
    in_maps = make_in_maps(inputs)
    res = run_bass_kernel_spmd(nc, in_maps, core_ids=list(range(N_CORES)))
    return assemble(res.results)


# revision 46
# speedup vs baseline: 1.0394x; 1.0394x over previous
"""MLA (multi-head latent attention) Trainium2 kernel.

Sharding: 8 cores = 2 (batch) x 4 (head groups of 4 heads).
Each core computes, for its batch b and heads [4g, 4g+4):
  latents kv_d/q_d (replicated within the batch group), per-head
  up-projections + RoPE, causal SDPA, and a partial o_proj
  out_core[o, q] = sum_{d in core's 512 head-dims} W_o[o, d] * y[d, q].
Host sums the 4 partials per batch (the all-reduce step of the hint,
performed at unshard time) and transposes to [S, H].

Key structure (v3):
  - Phase 1 streams xT once ([128,512] tiles, n-outer / k-inner) and
    computes kv_d, q_d and raw rope-k together in 6 PSUM banks, with
    packed per-k weight slices arriving just-in-time; k-RoPE runs
    per n-slice inside the loop.
  - SDPA runs per 512-query block at single 128-key-chunk granularity
    with a fine-grained diagonal (only the causal trapezoid is computed).
  - The softmax denominator is accumulated on the vector engine (bf16)
    across key chunks; a single ones-matmul per (head, q-block) reduces
    it across partitions.
  - PSUM->SBUF drains are split between the scalar and vector engines.

All matmuls run in bf16 with fp32 PSUM accumulation.
"""

import numpy as np
import ml_dtypes

import concourse.bass as bass
import concourse.mybir as mybir
import concourse.tile as tile
from concourse import bacc
from concourse._compat import get_trn_type
from concourse.bass_utils import run_bass_kernel_spmd

H = 2048
NH = 16
HD = 128           # head dim
RD = 64            # rotary dim
RH = 32            # rotary half
LAT = 256
B = 2
S = 2048
BASE = 10000.0
N_CORES = 8
HEADS_PER_CORE = 4
P = 128
NQB = S // 512     # 4 query blocks of 512
NKI = S // 128     # 16 key chunks of 128
SCALE = 1.0 / np.sqrt(float(HD))
EXP_BIAS = -4.0

BF16 = mybir.dt.bfloat16
F32 = mybir.dt.float32
_bf = ml_dtypes.bfloat16


def _mm(nc, out, lhsT, rhs, start, stop):
    nc.tensor.matmul(out, lhsT, rhs, start=start, stop=stop)


def build_program():
    nc = bacc.Bacc(
        get_trn_type() or "TRN2",
        target_bir_lowering=False,
        debug=False,
        num_devices=N_CORES,
    )

    xT = nc.declare_dram_parameter("xT", [H, S], BF16, isOutput=False)
    # packed phase-1 weights: per k-chunk [wrk(256) | wkvd(256) | wqd(256)]
    w_ph1 = nc.declare_dram_parameter("w_ph1", [P, 16, 768], BF16,
                                      isOutput=False)
    w_qc = nc.declare_dram_parameter("w_qc", [P, 2, 512], BF16, isOutput=False)
    w_kc = nc.declare_dram_parameter("w_kc", [P, 2, 256], BF16, isOutput=False)
    w_v = nc.declare_dram_parameter("w_v", [P, 2, 512], BF16, isOutput=False)
    w_o = nc.declare_dram_parameter("w_o", [P, 4, H], BF16, isOutput=False)
    cosA = nc.declare_dram_parameter("cosA", [P, S], BF16, isOutput=False)
    sinB = nc.declare_dram_parameter("sinB", [P, S], BF16, isOutput=False)
    masks = nc.declare_dram_parameter("masks", [P, P], BF16, isOutput=False)
    out = nc.declare_dram_parameter("out", [H, S], BF16, isOutput=True)

    Exp = mybir.ActivationFunctionType.Exp
    Mult = mybir.AluOpType.mult
    Add = mybir.AluOpType.add

    with tile.TileContext(nc) as tc:
        with (
            tc.tile_pool(name="main", bufs=1) as main,
        ):
            # -------- persistent small tensors --------
            cos_sb = main.tile([P, S], BF16, tag="cos", name="cos")
            sin_sb = main.tile([P, S], BF16, tag="sin", name="sin")
            mask_sb = main.tile([P, P], BF16, tag="mask", name="mask")
            ones_sb = main.tile([P, P], BF16, tag="ones", name="ones")
            nc.gpsimd.memset(ones_sb[:], 1.0)
            ebias_sb = main.tile([P, 1], F32, tag="ebias", name="ebias")
            nc.gpsimd.memset(ebias_sb[:], EXP_BIAS)
            wo_sb = main.tile([P, 4, H], BF16, tag="wo", name="wo")
            wqc_sb = main.tile([P, 2, 512], BF16, tag="wqc", name="wqc")
            wkc_sb = main.tile([P, 2, 256], BF16, tag="wkc", name="wkc")
            wv_sb = main.tile([P, 2, 512], BF16, tag="wv", name="wv")

            # -------- phase-1 outputs (latents + raw rope-k), bf16 --------
            kvd_sb = [main.tile([P, S], BF16, tag=f"kvd{m}", name=f"kvd{m}")
                      for m in range(2)]
            qd_sb = [main.tile([P, S], BF16, tag=f"qd{m}", name=f"qd{m}")
                     for m in range(2)]
            kraw = [main.tile([P, S], BF16, tag=f"kraw{p}", name=f"kraw{p}")
                    for p in range(2)]
            swpk = [main.tile([P, S], BF16, tag=f"swpk{p}", name=f"swpk{p}")
                    for p in range(2)]

            # phase-1 weights (packed)
            wph1_sb = main.tile([P, 16, 768], BF16, tag="wph1", name="wph1")

            # -------- DMA staging: critical-first --------
            # per-k packed weight slices, alternating between the scalar
            # and gpsimd queues (small transfers on two queues keep the
            # descriptor round-robin fair to the sync-queue x stream).
            for k in range(16):
                eng = nc.scalar if k % 2 == 0 else nc.gpsimd
                eng.dma_start(wph1_sb[:, k:k + 1, :], w_ph1[:, k:k + 1, :])
            # Phase-2/SDPA tensors are gated on n=0 completing (see n-loop).

            # -------- per-head q/k tiles (dims on partitions), v, y --------
            # even head h: rows [0:64] content, [64:128] rope
            # odd  head h: rows [0:64] rope,    [64:128] content
            qT = [main.tile([P, S], BF16, tag=f"qT{h}", name=f"qT{h}")
                  for h in range(4)]
            kT = [main.tile([P, S], BF16, tag=f"kT{h}", name=f"kT{h}")
                  for h in range(4)]
            v_sb = main.tile([P, NKI, 512], BF16, tag="v", name="v")
            y_sb = [main.tile([P, S], BF16, tag=f"y{h}", name=f"y{h}")
                    for h in range(4)]

            # -------- phase 1 (merged): stream xT once, n-outer/k-inner.
            # 6 PSUM banks per n-chunk: kr_g0, kr_g1, kvd0, kvd1, qd0, qd1.
            with tc.tile_pool(name="ps1", bufs=1, space="PSUM") as ps1:
                for n in range(4):
                    ns = slice(n * 512, (n + 1) * 512)
                    pb = [ps1.tile([P, 512], F32, tag=f"b{i}", name=f"p1_{i}")
                          for i in range(6)]
                    for k in range(16):
                        xtk = main.tile([P, 512], BF16, tag="xtk", name="xtk",
                                       bufs=10)
                        nc.sync.dma_start(
                            xtk[:], xT[k * 128:(k + 1) * 128, ns])
                        for i in range(6):
                            _mm(nc, pb[i][:],
                                wph1_sb[:, k, i * 128:(i + 1) * 128],
                                xtk[:], k == 0, k == 15)
                    # drain in next-iteration need order, split across engines
                    nc.vector.tensor_copy(kraw[0][:, ns], pb[0][:])
                    nc.scalar.copy(kraw[1][:, ns], pb[1][:])
                    nc.vector.tensor_copy(kvd_sb[0][:, ns], pb[2][:])
                    nc.scalar.copy(kvd_sb[1][:, ns], pb[3][:])
                    nc.vector.tensor_copy(qd_sb[0][:, ns], pb[4][:])
                    nc.scalar.copy(qd_sb[1][:, ns], pb[5][:])
                    if n == 0:
                        # phase-2/SDPA tensors: the tiny gpsimd copy reads an
                        # n=0 product, so the gpsimd queue actually WAITS for
                        # n=0 before issuing these loads (keeps HBM bandwidth
                        # for the phase-1 x/weight stream).
                        gate = main.tile([P, 8], BF16, tag="gate", name="gate",
                                        bufs=2)
                        nc.gpsimd.tensor_copy(gate[:], kvd_sb[0][:, 504:512])
                        nc.gpsimd.dma_start(cos_sb[:], cosA[:, :])
                        nc.gpsimd.dma_start(sin_sb[:], sinB[:, :])
                        nc.gpsimd.dma_start(mask_sb[:], masks[:, :])
                        nc.gpsimd.dma_start(wqc_sb[:], w_qc[:, :, :])
                        nc.gpsimd.dma_start(wkc_sb[:], w_kc[:, :, :])
                        nc.gpsimd.dma_start(wv_sb[:], w_v[:, :, :])
                    if n == 1:
                        gate = main.tile([P, 8], BF16, tag="gate", name="gate",
                                        bufs=2)
                        nc.gpsimd.tensor_copy(gate[:],
                                              kvd_sb[0][:, 1016:1024])
                        nc.gpsimd.dma_start(wo_sb[:], w_o[:, :, :])
                    if n > 0:
                        # rope on k for the PREVIOUS n-slice (cos/sin tables
                        # only arrive after the n=0 gate fires)
                        _rope_k(nc, n - 1, kraw, swpk, kT, cos_sb, sin_sb)
                _rope_k(nc, 3, kraw, swpk, kT, cos_sb, sin_sb)

            # -------- phase 2 + SDPA + o_proj (single PSUM epoch) --------
            # Phase-2 up-projections ride the "opj" PSUM ring per n-slice,
            # so SDPA q-block n never waits at a pool boundary for later
            # phase-2 work; ph2_chunk(n+1) is issued between SDPA heads and
            # o_proj to fill the softmax-tail bubble.
            with tc.tile_pool(name="ps3", bufs=1, space="PSUM") as ps3:
                swpq = main.tile([P, S], BF16, tag="swpq", name="swpq")

                def ph2_chunk(n):
                    ns = slice(n * 512, (n + 1) * 512)
                    # k content per pair: psum rows [0:64]=h0, [64:128]=h1
                    for p in range(2):
                        h0, h1 = 2 * p, 2 * p + 1
                        pt = ps3.tile([P, 512], F32, tag="opj", name="kcont",
                                      bufs=2)
                        for kc in range(2):
                            _mm(nc, pt[:],
                                wkc_sb[:, kc, p * 128:(p + 1) * 128],
                                kvd_sb[kc][:, ns], kc == 0, kc == 1)
                        nc.scalar.copy(kT[h0][0:64, ns], pt[0:64, :])
                        nc.scalar.copy(kT[h1][64:128, ns], pt[64:128, :])
                    # q combined up-proj + rope per head
                    for h in range(4):
                        r = 64 if h % 2 == 0 else 0
                        pt = ps3.tile([P, 512], F32, tag="opj", name="qcomb",
                                      bufs=2)
                        for kc in range(2):
                            _mm(nc, pt[:],
                                wqc_sb[:, kc, h * 128:(h + 1) * 128],
                                qd_sb[kc][:, ns], kc == 0, kc == 1)
                        nc.scalar.copy(qT[h][:, ns], pt[:])
                        nc.sync.dma_start(swpq[r:r + 32, ns],
                                          qT[h][r + 32:r + 64, ns])
                        nc.sync.dma_start(swpq[r + 32:r + 64, ns],
                                          qT[h][r:r + 32, ns])
                        nc.vector.tensor_mul(
                            qT[h][r:r + 64, ns], qT[h][r:r + 64, ns],
                            cos_sb[r:r + 64, ns]
                        )
                        nc.vector.tensor_mul(
                            swpq[r:r + 64, ns], swpq[r:r + 64, ns],
                            sin_sb[r:r + 64, ns]
                        )
                        nc.vector.tensor_add(
                            qT[h][r:r + 64, ns], qT[h][r:r + 64, ns],
                            swpq[r:r + 64, ns]
                        )
                    # v chunks for this n-slice (all 4 heads x 128 dims)
                    for s16 in range(4 * n, 4 * n + 4):
                        pt = ps3.tile([P, 512], F32, tag="opj", name="vps",
                                      bufs=2)
                        for kc in range(2):
                            _mm(nc, pt[:],
                                kvd_sb[kc][:, s16 * 128:(s16 + 1) * 128],
                                wv_sb[:, kc, 0:512], kc == 0, kc == 1)
                        nc.vector.tensor_copy(v_sb[:, s16, :], pt[:])

                def emit_oproj(qbp, ocs):
                    """Partial o_proj for q-block ``qbp`` (a few oc chunks)."""
                    qsp = slice(qbp * 512, (qbp + 1) * 512)
                    for oc in ocs:
                        opt_ = ps3.tile([P, 512], F32, tag="opj", name="opj",
                                        bufs=2)
                        for hk in range(4):
                            _mm(nc, opt_[:],
                                wo_sb[:, hk, oc * 128:(oc + 1) * 128],
                                y_sb[hk][:, qsp], hk == 0, hk == 3)
                        osb = main.tile([P, 512], BF16, tag="osb", name="osb",
                                        bufs=4)
                        if oc % 2 == 0:
                            nc.scalar.copy(osb[:], opt_[:])
                        else:
                            nc.vector.tensor_copy(osb[:], opt_[:])
                        nc.sync.dma_start(out[oc * 128:(oc + 1) * 128, qsp],
                                          osb[:])

                ph2_chunk(0)
                for qb in range(NQB):
                    qs = slice(qb * 512, (qb + 1) * 512)
                    nod = 4 * qb          # off-diagonal chunks (pairs)
                    for h in range(4):
                        hv = h * 128
                        yps = ps3.tile([P, 512], F32, tag="ypv", name="ypv",
                                       bufs=2)
                        # bf16 denominator accumulator; the first exp writes
                        # it directly (doubles as that pair's prb)
                        tacc = main.tile([P, 1024], BF16, tag="tacc",
                                        name="tacc", bufs=2)
                        # off-diagonal chunk pairs: full width
                        for g in range(nod // 2):
                            scps = ps3.tile([P, 1024], F32, tag="sc",
                                            name="sc", bufs=2)
                            for j in range(2):
                                ki = 2 * g + j
                                _mm(nc, scps[:, j * 512:(j + 1) * 512],
                                    kT[h][:, ki * 128:(ki + 1) * 128],
                                    qT[h][:, qs], True, True)
                            if g == 0:
                                prb = tacc
                            else:
                                prb = main.tile([P, 1024], BF16, tag="prb",
                                               name="prb", bufs=4)
                            nc.scalar.activation(
                                prb[:], scps[:], Exp,
                                bias=ebias_sb[:], scale=SCALE
                            )
                            if g > 0:
                                nc.vector.tensor_add(tacc[:], tacc[:], prb[:])
                            for j in range(2):
                                ki = 2 * g + j
                                _mm(nc, yps[:],
                                    v_sb[:, ki, hv:hv + 128],
                                    prb[:, j * 512:(j + 1) * 512],
                                    ki == 0, False)
                        # diagonal chunks: shrinking width + triangular mask
                        for l in range(4):
                            ki = nod + l
                            c0 = 128 * l
                            cs = slice(c0, 512)
                            scps = ps3.tile([P, 1024], F32, tag="sc",
                                            name="sc", bufs=2)
                            _mm(nc, scps[:, cs],
                                kT[h][:, ki * 128:(ki + 1) * 128],
                                qT[h][:, qb * 512 + c0:(qb + 1) * 512],
                                True, True)
                            if qb == 0 and l == 0:
                                prb = tacc
                            else:
                                prb = main.tile([P, 1024], BF16, tag="prb",
                                               name="prb", bufs=4)
                            nc.scalar.activation(
                                prb[:, cs], scps[:, cs], Exp,
                                bias=ebias_sb[:], scale=SCALE
                            )
                            nc.vector.tensor_mul(
                                prb[:, c0:c0 + 128],
                                prb[:, c0:c0 + 128], mask_sb[:]
                            )
                            if not (qb == 0 and l == 0):
                                nc.vector.tensor_add(
                                    tacc[:, cs], tacc[:, cs], prb[:, cs])
                            # AV: start=True clears has_written for the WHOLE
                            # bank, so it appears exactly once (ki==0, which
                            # is always full width).
                            _mm(nc, yps[:, cs],
                                v_sb[:, ki, hv:hv + 128],
                                prb[:, cs], ki == 0, l == 3)
                        # reduce denominator across partitions: accumulate
                        # both tacc halves into one PSUM region (no DVE fold
                        # on the critical path)
                        rsps = ps3.tile([P, 1024], F32, tag="sc", name="rs",
                                        bufs=2)
                        _mm(nc, rsps[:, 0:512], ones_sb[:], tacc[:, 0:512],
                            True, qb == 0)
                        if qb > 0:
                            _mm(nc, rsps[:, 0:512], ones_sb[:],
                                tacc[:, 512:1024], False, True)
                        rcp = main.tile([P, 512], F32, tag="rcp", name="rcp",
                                       bufs=2)
                        nc.vector.reciprocal_approx_fast(rcp[:], rsps[:, 0:512])
                        nc.vector.tensor_mul(y_sb[h][:, qs], yps[:], rcp[:])
                        if qb > 0:
                            # interleave the previous q-block's o_proj: ready
                            # tensor work that absorbs exp-paced score stalls
                            emit_oproj(qb - 1, range(4 * h, 4 * h + 4))
                    if qb < 3:
                        # next n-slice's up-projections fill the tail bubble
                        ph2_chunk(qb + 1)
                emit_oproj(3, range(16))

    nc.compile()
    return nc


def _rope_k(nc, n, kraw, swpk, kT, cos_sb, sin_sb):
    """Apply RoPE to the raw rope-k slices for n-chunk ``n``.

    kraw[p] rows [0:64] = h1 rope dims, [64:128] = h0 rope dims.
    swp DMAs ride the sync queue; the muls/adds ride the vector engine
    (both are otherwise idle mid-phase-1).
    """
    ns = slice(n * 512, (n + 1) * 512)
    for p in range(2):
        h0, h1 = 2 * p, 2 * p + 1
        sw = swpk[p]
        nc.sync.dma_start(sw[0:32, ns], kraw[p][32:64, ns])
        nc.sync.dma_start(sw[32:64, ns], kraw[p][0:32, ns])
        nc.sync.dma_start(sw[64:96, ns], kraw[p][96:128, ns])
        nc.sync.dma_start(sw[96:128, ns], kraw[p][64:96, ns])
        nc.vector.tensor_mul(sw[:, ns], sw[:, ns], sin_sb[:, ns])
        nc.vector.tensor_mul(
            kT[h1][0:64, ns], kraw[p][0:64, ns], cos_sb[0:64, ns]
        )
        nc.vector.tensor_add(
            kT[h1][0:64, ns], kT[h1][0:64, ns], sw[0:64, ns]
        )
        nc.vector.tensor_mul(
            kT[h0][64:128, ns], kraw[p][64:128, ns], cos_sb[64:128, ns]
        )
        nc.vector.tensor_add(
            kT[h0][64:128, ns], kT[h0][64:128, ns], sw[64:128, ns]
        )


_NC = None


def _get_nc():
    global _NC
    if _NC is None:
        _NC = build_program()
    return _NC


def _rope_tables():
    """cosA/sinB [128, S]: 32-row frequency pattern tiled 4x.
    sinB sign: rows [0:32] of each 64-block -> -sin, rows [32:64] -> +sin."""
    inv_freq = 1.0 / (BASE ** (np.arange(0, RD, 2, dtype=np.float32) / RD))  # [32]
    pos = np.arange(S, dtype=np.float32)
    ang = inv_freq[:, None] * pos[None, :]              # [32, S]
    cos1, sin1 = np.cos(ang), np.sin(ang)
    cosA = np.tile(cos1, (4, 1))                        # [128, S]
    sinB = np.concatenate([-sin1, sin1, -sin1, sin1], axis=0)
    return cosA.astype(_bf), sinB.astype(_bf)


def _mask_tiles():
    """mask[k, q] = 1.0 if q >= k else 0 (bf16, [128,128] triangular)."""
    k = np.arange(P)[:, None]
    q = np.arange(P)[None, :]
    return (q >= k).astype(np.float32).astype(_bf)


def _prep_core_inputs(c, x, W_kv_d, W_q_d, W_k_u, W_q_u, W_v_u, W_rope_k, W_rope_q,
                      W_o, cosA, sinB, masks):
    b = c // 4
    hg = c % 4
    heads = [4 * hg + j for j in range(HEADS_PER_CORE)]

    def tile_pmaj(w):
        # [ko*128, m] -> [128, ko, m] partition-major for contiguous DMA
        ko = w.shape[0] // P
        return np.ascontiguousarray(
            w.reshape(ko, P, w.shape[1]).transpose(1, 0, 2))

    xT = np.ascontiguousarray(x[b].T).astype(_bf)                  # [H, S]
    w_kvd = tile_pmaj(np.ascontiguousarray(W_kv_d.T).astype(_bf))
    w_qd = tile_pmaj(np.ascontiguousarray(W_q_d.T).astype(_bf))

    # w_rk: per pair, rows [h1 rope dims | h0 rope dims], then transpose
    blocks = []
    for p in range(2):
        g0, g1 = heads[2 * p], heads[2 * p + 1]
        blocks.append(W_rope_k[g1 * RD:(g1 + 1) * RD, :])
        blocks.append(W_rope_k[g0 * RD:(g0 + 1) * RD, :])
    w_rk = tile_pmaj(np.ascontiguousarray(np.concatenate(blocks, axis=0).T).astype(_bf))

    # packed phase-1 weights: per k-chunk [wrk | wkvd | wqd]
    w_ph1 = np.ascontiguousarray(
        np.concatenate([w_rk, w_kvd, w_qd], axis=2))

    # w_qc: per local head 128 cols: even -> [content|rope], odd -> [rope|content]
    cols = []
    for j, g in enumerate(heads):
        c_blk = W_q_u[g * RD:(g + 1) * RD, :].T       # [LAT, 64]
        r_blk = W_rope_q[g * RD:(g + 1) * RD, :].T    # [LAT, 64]
        cols.extend([c_blk, r_blk] if j % 2 == 0 else [r_blk, c_blk])
    w_qc = tile_pmaj(np.ascontiguousarray(np.concatenate(cols, axis=1)).astype(_bf))

    # w_kc: per pair 128 cols: [h0 content | h1 content]
    cols = []
    for p in range(2):
        g0, g1 = heads[2 * p], heads[2 * p + 1]
        cols.append(W_k_u[g0 * RD:(g0 + 1) * RD, :].T)
        cols.append(W_k_u[g1 * RD:(g1 + 1) * RD, :].T)
    w_kc = tile_pmaj(np.ascontiguousarray(np.concatenate(cols, axis=1)).astype(_bf))

    # w_v: per head 128 cols, heads in order (512 total)
    cols = [W_v_u[g * HD:(g + 1) * HD, :].T for g in heads]
    w_v = tile_pmaj(np.ascontiguousarray(np.concatenate(cols, axis=1)).astype(_bf))

    d0 = heads[0] * HD
    w_o = tile_pmaj(np.ascontiguousarray(W_o[:, d0:d0 + 512].T).astype(_bf))

    return {
        "xT": xT, "w_ph1": w_ph1, "w_qc": w_qc,
        "w_kc": w_kc, "w_v": w_v, "w_o": w_o, "cosA": cosA, "sinB": sinB,
        "masks": masks,
    }


def make_in_maps(inputs):
    x = np.asarray(inputs["hidden_states"], dtype=np.float32)
    ws = {k: np.asarray(inputs[k], dtype=np.float32)
          for k in ("W_kv_d", "W_q_d", "W_k_u", "W_q_u", "W_v_u", "W_rope_k",
                    "W_rope_q", "W_o")}
    cosA, sinB = _rope_tables()
    masks = _mask_tiles()
    return [
        _prep_core_inputs(c, x, ws["W_kv_d"], ws["W_q_d"], ws["W_k_u"],
                          ws["W_q_u"], ws["W_v_u"], ws["W_rope_k"],
                          ws["W_rope_q"], ws["W_o"], cosA, sinB, masks)
        for c in range(N_CORES)
    ]


def assemble(results):
    """results: list of 8 dicts with 'out' [H, S] partials (transposed)."""
    full = np.empty((B, S, H), dtype=np.float32)
    for b in range(B):
        acc = results[4 * b]["out"].astype(np.float32)
        for g in range(1, 4):
            acc = acc + results[4 * b + g]["out"]
        full[b] = acc.T
    return full


def kernel(**inputs):
    nc = _get_nc()
    in_maps = make_in_maps(inputs)
    res = run_bass_kernel_spmd(nc, in_maps, core_ids=list(range(N_CORES)))
    return assemble(res.results)


# revision 47
# speedup vs baseline: 1.0484x; 1.0087x over previous
"""MLA (multi-head latent attention) Trainium2 kernel.

Sharding: 8 cores = 2 (batch) x 4 (head groups of 4 heads).
Each core computes, for its batch b and heads [4g, 4g+4):
  latents kv_d/q_d (replicated within the batch group), per-head
  up-projections + RoPE, causal SDPA, and a partial o_proj
  out_core[o, q] = sum_{d in core's 512 head-dims} W_o[o, d] * y[d, q].
Host sums the 4 partials per batch (the all-reduce step of the hint,
performed at unshard time) and transposes to [S, H].

Key structure (v3):
  - Phase 1 streams xT once ([128,512] tiles, n-outer / k-inner) and
    computes kv_d, q_d and raw rope-k together in 6 PSUM banks, with
    packed per-k weight slices arriving just-in-time; k-RoPE runs
    per n-slice inside the loop.
  - SDPA runs per 512-query block at single 128-key-chunk granularity
    with a fine-grained diagonal (only the causal trapezoid is computed).
  - The softmax denominator is accumulated on the vector engine (bf16)
    across key chunks; a single ones-matmul per (head, q-block) reduces
    it across partitions.
  - PSUM->SBUF drains are split between the scalar and vector engines.

All matmuls run in bf16 with fp32 PSUM accumulation.
"""

import numpy as np
import ml_dtypes

import concourse.bass as bass
import concourse.mybir as mybir
import concourse.tile as tile
from concourse import bacc
from concourse._compat import get_trn_type
from concourse.bass_utils import run_bass_kernel_spmd

H = 2048
NH = 16
HD = 128           # head dim
RD = 64            # rotary dim
RH = 32            # rotary half
LAT = 256
B = 2
S = 2048
BASE = 10000.0
N_CORES = 8
HEADS_PER_CORE = 4
P = 128
NQB = S // 512     # 4 query blocks of 512
NKI = S // 128     # 16 key chunks of 128
SCALE = 1.0 / np.sqrt(float(HD))
EXP_BIAS = -4.0

BF16 = mybir.dt.bfloat16
F32 = mybir.dt.float32
_bf = ml_dtypes.bfloat16


def _mm(nc, out, lhsT, rhs, start, stop):
    nc.tensor.matmul(out, lhsT, rhs, start=start, stop=stop)


def build_program():
    nc = bacc.Bacc(
        get_trn_type() or "TRN2",
        target_bir_lowering=False,
        debug=False,
        num_devices=N_CORES,
    )

    xT = nc.declare_dram_parameter("xT", [H, S], BF16, isOutput=False)
    # packed phase-1 weights: per k-chunk [wrk(256) | wkvd(256) | wqd(256)]
    w_ph1 = nc.declare_dram_parameter("w_ph1", [P, 16, 768], BF16,
                                      isOutput=False)
    w_qc = nc.declare_dram_parameter("w_qc", [P, 2, 512], BF16, isOutput=False)
    w_kc = nc.declare_dram_parameter("w_kc", [P, 2, 256], BF16, isOutput=False)
    w_v = nc.declare_dram_parameter("w_v", [P, 2, 512], BF16, isOutput=False)
    w_o = nc.declare_dram_parameter("w_o", [P, 4, H], BF16, isOutput=False)
    cosA = nc.declare_dram_parameter("cosA", [P, S], BF16, isOutput=False)
    sinB = nc.declare_dram_parameter("sinB", [P, S], BF16, isOutput=False)
    masks = nc.declare_dram_parameter("masks", [P, P], BF16, isOutput=False)
    out = nc.declare_dram_parameter("out", [H, S], BF16, isOutput=True)

    Exp = mybir.ActivationFunctionType.Exp
    Mult = mybir.AluOpType.mult
    Add = mybir.AluOpType.add

    with tile.TileContext(nc) as tc:
        with (
            tc.tile_pool(name="main", bufs=1) as main,
        ):
            # -------- persistent small tensors --------
            cos_sb = main.tile([P, S], BF16, tag="cos", name="cos")
            sin_sb = main.tile([P, S], BF16, tag="sin", name="sin")
            mask_sb = main.tile([P, P], BF16, tag="mask", name="mask")
            ones_sb = main.tile([P, P], BF16, tag="ones", name="ones")
            nc.gpsimd.memset(ones_sb[:], 1.0)
            ebias_sb = main.tile([P, 1], F32, tag="ebias", name="ebias")
            nc.gpsimd.memset(ebias_sb[:], EXP_BIAS)
            wo_sb = main.tile([P, 4, H], BF16, tag="wo", name="wo")
            wqc_sb = main.tile([P, 2, 512], BF16, tag="wqc", name="wqc")
            wkc_sb = main.tile([P, 2, 256], BF16, tag="wkc", name="wkc")
            wv_sb = main.tile([P, 2, 512], BF16, tag="wv", name="wv")

            # -------- phase-1 outputs (latents + raw rope-k), bf16 --------
            kvd_sb = [main.tile([P, S], BF16, tag=f"kvd{m}", name=f"kvd{m}")
                      for m in range(2)]
            qd_sb = [main.tile([P, S], BF16, tag=f"qd{m}", name=f"qd{m}")
                     for m in range(2)]
            kraw = [main.tile([P, S], BF16, tag=f"kraw{p}", name=f"kraw{p}")
                    for p in range(2)]
            swpk = [main.tile([P, S], BF16, tag=f"swpk{p}", name=f"swpk{p}")
                    for p in range(2)]

            # phase-1 weights (packed)
            wph1_sb = main.tile([P, 16, 768], BF16, tag="wph1", name="wph1")

            # -------- DMA staging: critical-first --------
            # per-k packed weight slices, alternating between the scalar
            # and gpsimd queues (small transfers on two queues keep the
            # descriptor round-robin fair to the sync-queue x stream).
            for k in range(16):
                eng = nc.scalar if k % 2 == 0 else nc.gpsimd
                eng.dma_start(wph1_sb[:, k:k + 1, :], w_ph1[:, k:k + 1, :])
            # Phase-2/SDPA tensors are gated on n=0 completing (see n-loop).

            # -------- per-head q/k tiles (dims on partitions), v, y --------
            # even head h: rows [0:64] content, [64:128] rope
            # odd  head h: rows [0:64] rope,    [64:128] content
            qT = [main.tile([P, S], BF16, tag=f"qT{h}", name=f"qT{h}")
                  for h in range(4)]
            kT = [main.tile([P, S], BF16, tag=f"kT{h}", name=f"kT{h}")
                  for h in range(4)]
            v_sb = main.tile([P, NKI, 512], BF16, tag="v", name="v")
            y_sb = [main.tile([P, S], BF16, tag=f"y{h}", name=f"y{h}")
                    for h in range(4)]

            # -------- phase 1 (merged): stream xT once, n-outer/k-inner.
            # 6 PSUM banks per n-chunk: kr_g0, kr_g1, kvd0, kvd1, qd0, qd1.
            with tc.tile_pool(name="ps1", bufs=1, space="PSUM") as ps1:
                for n in range(4):
                    ns = slice(n * 512, (n + 1) * 512)
                    pb = [ps1.tile([P, 512], F32, tag=f"b{i}", name=f"p1_{i}")
                          for i in range(6)]
                    for k in range(16):
                        xtk = main.tile([P, 512], BF16, tag="xtk", name="xtk",
                                       bufs=10)
                        nc.sync.dma_start(
                            xtk[:], xT[k * 128:(k + 1) * 128, ns])
                        for i in range(6):
                            _mm(nc, pb[i][:],
                                wph1_sb[:, k, i * 128:(i + 1) * 128],
                                xtk[:], k == 0, k == 15)
                    # drain in next-iteration need order, split across engines
                    nc.vector.tensor_copy(kraw[0][:, ns], pb[0][:])
                    nc.scalar.copy(kraw[1][:, ns], pb[1][:])
                    nc.vector.tensor_copy(kvd_sb[0][:, ns], pb[2][:])
                    nc.scalar.copy(kvd_sb[1][:, ns], pb[3][:])
                    nc.vector.tensor_copy(qd_sb[0][:, ns], pb[4][:])
                    nc.scalar.copy(qd_sb[1][:, ns], pb[5][:])
                    if n == 0:
                        # phase-2/SDPA tensors: the tiny gpsimd copy reads an
                        # n=0 product, so the gpsimd queue actually WAITS for
                        # n=0 before issuing these loads (keeps HBM bandwidth
                        # for the phase-1 x/weight stream).
                        gate = main.tile([P, 8], BF16, tag="gate", name="gate",
                                        bufs=2)
                        nc.gpsimd.tensor_copy(gate[:], kvd_sb[0][:, 504:512])
                        nc.gpsimd.dma_start(cos_sb[:], cosA[:, :])
                        nc.gpsimd.dma_start(sin_sb[:], sinB[:, :])
                        nc.gpsimd.dma_start(mask_sb[:], masks[:, :])
                        nc.gpsimd.dma_start(wqc_sb[:], w_qc[:, :, :])
                        nc.gpsimd.dma_start(wkc_sb[:], w_kc[:, :, :])
                        nc.gpsimd.dma_start(wv_sb[:], w_v[:, :, :])
                    if n == 1:
                        gate = main.tile([P, 8], BF16, tag="gate", name="gate",
                                        bufs=2)
                        nc.gpsimd.tensor_copy(gate[:],
                                              kvd_sb[0][:, 1016:1024])
                        nc.gpsimd.dma_start(wo_sb[:], w_o[:, :, :])
                    if n > 0:
                        # rope on k for the PREVIOUS n-slice (cos/sin tables
                        # only arrive after the n=0 gate fires)
                        _rope_k(nc, n - 1, kraw, swpk, kT, cos_sb, sin_sb)
                _rope_k(nc, 3, kraw, swpk, kT, cos_sb, sin_sb)

            # -------- phase 2 + SDPA + o_proj (single PSUM epoch) --------
            # Phase-2 up-projections ride the "opj" PSUM ring per n-slice,
            # so SDPA q-block n never waits at a pool boundary for later
            # phase-2 work; ph2_chunk(n+1) is issued between SDPA heads and
            # o_proj to fill the softmax-tail bubble.
            with tc.tile_pool(name="ps3", bufs=1, space="PSUM") as ps3:
                swpq = main.tile([P, S], BF16, tag="swpq", name="swpq")

                def ph2_chunk(n):
                    ns = slice(n * 512, (n + 1) * 512)
                    # k content per pair: psum rows [0:64]=h0, [64:128]=h1
                    for p in range(2):
                        h0, h1 = 2 * p, 2 * p + 1
                        pt = ps3.tile([P, 512], F32, tag="opj", name="kcont",
                                      bufs=2)
                        for kc in range(2):
                            _mm(nc, pt[:],
                                wkc_sb[:, kc, p * 128:(p + 1) * 128],
                                kvd_sb[kc][:, ns], kc == 0, kc == 1)
                        nc.scalar.copy(kT[h0][0:64, ns], pt[0:64, :])
                        nc.scalar.copy(kT[h1][64:128, ns], pt[64:128, :])
                    # q combined up-proj + rope per head
                    for h in range(4):
                        r = 64 if h % 2 == 0 else 0
                        pt = ps3.tile([P, 512], F32, tag="opj", name="qcomb",
                                      bufs=2)
                        for kc in range(2):
                            _mm(nc, pt[:],
                                wqc_sb[:, kc, h * 128:(h + 1) * 128],
                                qd_sb[kc][:, ns], kc == 0, kc == 1)
                        nc.scalar.copy(qT[h][:, ns], pt[:])
                        nc.sync.dma_start(swpq[r:r + 32, ns],
                                          qT[h][r + 32:r + 64, ns])
                        nc.sync.dma_start(swpq[r + 32:r + 64, ns],
                                          qT[h][r:r + 32, ns])
                        nc.vector.tensor_mul(
                            qT[h][r:r + 64, ns], qT[h][r:r + 64, ns],
                            cos_sb[r:r + 64, ns]
                        )
                        nc.vector.tensor_mul(
                            swpq[r:r + 64, ns], swpq[r:r + 64, ns],
                            sin_sb[r:r + 64, ns]
                        )
                        nc.vector.tensor_add(
                            qT[h][r:r + 64, ns], qT[h][r:r + 64, ns],
                            swpq[r:r + 64, ns]
                        )
                    # v chunks for this n-slice (all 4 heads x 128 dims)
                    for s16 in range(4 * n, 4 * n + 4):
                        pt = ps3.tile([P, 512], F32, tag="opj", name="vps",
                                      bufs=2)
                        for kc in range(2):
                            _mm(nc, pt[:],
                                kvd_sb[kc][:, s16 * 128:(s16 + 1) * 128],
                                wv_sb[:, kc, 0:512], kc == 0, kc == 1)
                        nc.vector.tensor_copy(v_sb[:, s16, :], pt[:])

                def emit_oproj(qbp, ocs):
                    """Partial o_proj for q-block ``qbp`` (a few oc chunks)."""
                    qsp = slice(qbp * 512, (qbp + 1) * 512)
                    for oc in ocs:
                        opt_ = ps3.tile([P, 512], F32, tag="opj", name="opj",
                                        bufs=2)
                        for hk in range(4):
                            _mm(nc, opt_[:],
                                wo_sb[:, hk, oc * 128:(oc + 1) * 128],
                                y_sb[hk][:, qsp], hk == 0, hk == 3)
                        osb = main.tile([P, 512], BF16, tag="osb", name="osb",
                                        bufs=4)
                        if oc % 2 == 0:
                            nc.scalar.copy(osb[:], opt_[:])
                        else:
                            nc.vector.tensor_copy(osb[:], opt_[:])
                        nc.sync.dma_start(out[oc * 128:(oc + 1) * 128, qsp],
                                          osb[:])

                ph2_chunk(0)
                for qb in range(NQB):
                    qs = slice(qb * 512, (qb + 1) * 512)
                    nod = 4 * qb          # off-diagonal chunks (pairs)
                    for h in range(4):
                        hv = h * 128
                        yps = ps3.tile([P, 512], F32, tag="ypv", name="ypv",
                                       bufs=2)
                        # bf16 denominator accumulator; the first exp writes
                        # it directly (doubles as that pair's prb)
                        tacc = main.tile([P, 1024], BF16, tag="tacc",
                                        name="tacc", bufs=2)
                        # off-diagonal chunk pairs: full width
                        for g in range(nod // 2):
                            scps = ps3.tile([P, 1024], F32, tag="sc",
                                            name="sc", bufs=2)
                            for j in range(2):
                                ki = 2 * g + j
                                _mm(nc, scps[:, j * 512:(j + 1) * 512],
                                    kT[h][:, ki * 128:(ki + 1) * 128],
                                    qT[h][:, qs], True, True)
                            if g == 0:
                                prb = tacc
                            else:
                                prb = main.tile([P, 1024], BF16, tag="prb",
                                               name="prb", bufs=6)
                            nc.scalar.activation(
                                prb[:], scps[:], Exp,
                                bias=ebias_sb[:], scale=SCALE
                            )
                            if g > 0:
                                nc.vector.tensor_add(tacc[:], tacc[:], prb[:])
                            for j in range(2):
                                ki = 2 * g + j
                                _mm(nc, yps[:],
                                    v_sb[:, ki, hv:hv + 128],
                                    prb[:, j * 512:(j + 1) * 512],
                                    ki == 0, False)
                        # diagonal chunks: shrinking width + triangular mask
                        for l in range(4):
                            ki = nod + l
                            c0 = 128 * l
                            cs = slice(c0, 512)
                            scps = ps3.tile([P, 1024], F32, tag="sc",
                                            name="sc", bufs=2)
                            _mm(nc, scps[:, cs],
                                kT[h][:, ki * 128:(ki + 1) * 128],
                                qT[h][:, qb * 512 + c0:(qb + 1) * 512],
                                True, True)
                            if qb == 0 and l == 0:
                                prb = tacc
                            else:
                                prb = main.tile([P, 1024], BF16, tag="prb",
                                               name="prb", bufs=6)
                            nc.scalar.activation(
                                prb[:, cs], scps[:, cs], Exp,
                                bias=ebias_sb[:], scale=SCALE
                            )
                            nc.vector.tensor_mul(
                                prb[:, c0:c0 + 128],
                                prb[:, c0:c0 + 128], mask_sb[:]
                            )
                            if not (qb == 0 and l == 0):
                                nc.vector.tensor_add(
                                    tacc[:, cs], tacc[:, cs], prb[:, cs])
                            # AV: start=True clears has_written for the WHOLE
                            # bank, so it appears exactly once (ki==0, which
                            # is always full width).
                            _mm(nc, yps[:, cs],
                                v_sb[:, ki, hv:hv + 128],
                                prb[:, cs], ki == 0, l == 3)
                        # reduce denominator across partitions: accumulate
                        # both tacc halves into one PSUM region (no DVE fold
                        # on the critical path)
                        rsps = ps3.tile([P, 1024], F32, tag="sc", name="rs",
                                        bufs=2)
                        _mm(nc, rsps[:, 0:512], ones_sb[:], tacc[:, 0:512],
                            True, qb == 0)
                        if qb > 0:
                            _mm(nc, rsps[:, 0:512], ones_sb[:],
                                tacc[:, 512:1024], False, True)
                        rcp = main.tile([P, 512], F32, tag="rcp", name="rcp",
                                       bufs=2)
                        nc.vector.reciprocal_approx_fast(rcp[:], rsps[:, 0:512])
                        nc.vector.tensor_mul(y_sb[h][:, qs], yps[:], rcp[:])
                        if qb > 0:
                            # interleave the previous q-block's o_proj: ready
                            # tensor work that absorbs exp-paced score stalls
                            emit_oproj(qb - 1, range(4 * h, 4 * h + 4))
                    if qb < 3:
                        # next n-slice's up-projections fill the tail bubble
                        ph2_chunk(qb + 1)
                emit_oproj(3, range(16))

    nc.compile()
    return nc


def _rope_k(nc, n, kraw, swpk, kT, cos_sb, sin_sb):
    """Apply RoPE to the raw rope-k slices for n-chunk ``n``.

    kraw[p] rows [0:64] = h1 rope dims, [64:128] = h0 rope dims.
    swp DMAs ride the sync queue; the muls/adds ride the vector engine
    (both are otherwise idle mid-phase-1).
    """
    ns = slice(n * 512, (n + 1) * 512)
    for p in range(2):
        h0, h1 = 2 * p, 2 * p + 1
        sw = swpk[p]
        nc.sync.dma_start(sw[0:32, ns], kraw[p][32:64, ns])
        nc.sync.dma_start(sw[32:64, ns], kraw[p][0:32, ns])
        nc.sync.dma_start(sw[64:96, ns], kraw[p][96:128, ns])
        nc.sync.dma_start(sw[96:128, ns], kraw[p][64:96, ns])
        nc.vector.tensor_mul(sw[:, ns], sw[:, ns], sin_sb[:, ns])
        nc.vector.tensor_mul(
            kT[h1][0:64, ns], kraw[p][0:64, ns], cos_sb[0:64, ns]
        )
        nc.vector.tensor_add(
            kT[h1][0:64, ns], kT[h1][0:64, ns], sw[0:64, ns]
        )
        nc.vector.tensor_mul(
            kT[h0][64:128, ns], kraw[p][64:128, ns], cos_sb[64:128, ns]
        )
        nc.vector.tensor_add(
            kT[h0][64:128, ns], kT[h0][64:128, ns], sw[64:128, ns]
        )


_NC = None


def _get_nc():
    global _NC
    if _NC is None:
        _NC = build_program()
    return _NC


def _rope_tables():
    """cosA/sinB [128, S]: 32-row frequency pattern tiled 4x.
    sinB sign: rows [0:32] of each 64-block -> -sin, rows [32:64] -> +sin."""
    inv_freq = 1.0 / (BASE ** (np.arange(0, RD, 2, dtype=np.float32) / RD))  # [32]
    pos = np.arange(S, dtype=np.float32)
    ang = inv_freq[:, None] * pos[None, :]              # [32, S]
    cos1, sin1 = np.cos(ang), np.sin(ang)
    cosA = np.tile(cos1, (4, 1))                        # [128, S]
    sinB = np.concatenate([-sin1, sin1, -sin1, sin1], axis=0)
    return cosA.astype(_bf), sinB.astype(_bf)


def _mask_tiles():
    """mask[k, q] = 1.0 if q >= k else 0 (bf16, [128,128] triangular)."""
    k = np.arange(P)[:, None]
    q = np.arange(P)[None, :]
    return (q >= k).astype(np.float32).astype(_bf)


def _prep_core_inputs(c, x, W_kv_d, W_q_d, W_k_u, W_q_u, W_v_u, W_rope_k, W_rope_q,
                      W_o, cosA, sinB, masks):
    b = c // 4
    hg = c % 4
    heads = [4 * hg + j for j in range(HEADS_PER_CORE)]

    def tile_pmaj(w):
        # [ko*128, m] -> [128, ko, m] partition-major for contiguous DMA
        ko = w.shape[0] // P
        return np.ascontiguousarray(
            w.reshape(ko, P, w.shape[1]).transpose(1, 0, 2))

    xT = np.ascontiguousarray(x[b].T).astype(_bf)                  # [H, S]
    w_kvd = tile_pmaj(np.ascontiguousarray(W_kv_d.T).astype(_bf))
    w_qd = tile_pmaj(np.ascontiguousarray(W_q_d.T).astype(_bf))

    # w_rk: per pair, rows [h1 rope dims | h0 rope dims], then transpose
    blocks = []
    for p in range(2):
        g0, g1 = heads[2 * p], heads[2 * p + 1]
        blocks.append(W_rope_k[g1 * RD:(g1 + 1) * RD, :])
        blocks.append(W_rope_k[g0 * RD:(g0 + 1) * RD, :])
    w_rk = tile_pmaj(np.ascontiguousarray(np.concatenate(blocks, axis=0).T).astype(_bf))

    # packed phase-1 weights: per k-chunk [wrk | wkvd | wqd]
    w_ph1 = np.ascontiguousarray(
        np.concatenate([w_rk, w_kvd, w_qd], axis=2))

    # w_qc: per local head 128 cols: even -> [content|rope], odd -> [rope|content]
    cols = []
    for j, g in enumerate(heads):
        c_blk = W_q_u[g * RD:(g + 1) * RD, :].T       # [LAT, 64]
        r_blk = W_rope_q[g * RD:(g + 1) * RD, :].T    # [LAT, 64]
        cols.extend([c_blk, r_blk] if j % 2 == 0 else [r_blk, c_blk])
    w_qc = tile_pmaj(np.ascontiguousarray(np.concatenate(cols, axis=1)).astype(_bf))

    # w_kc: per pair 128 cols: [h0 content | h1 content]
    cols = []
    for p in range(2):
        g0, g1 = heads[2 * p], heads[2 * p + 1]
        cols.append(W_k_u[g0 * RD:(g0 + 1) * RD, :].T)
        cols.append(W_k_u[g1 * RD:(g1 + 1) * RD, :].T)
    w_kc = tile_pmaj(np.ascontiguousarray(np.concatenate(cols, axis=1)).astype(_bf))

    # w_v: per head 128 cols, heads in order (512 total)
    cols = [W_v_u[g * HD:(g + 1) * HD, :].T for g in heads]
    w_v = tile_pmaj(np.ascontiguousarray(np.concatenate(cols, axis=1)).astype(_bf))

    d0 = heads[0] * HD
    w_o = tile_pmaj(np.ascontiguousarray(W_o[:, d0:d0 + 512].T).astype(_bf))

    return {
        "xT": xT, "w_ph1": w_ph1, "w_qc": w_qc,
        "w_kc": w_kc, "w_v": w_v, "w_o": w_o, "cosA": cosA, "sinB": sinB,
        "masks": masks,
    }


def make_in_maps(inputs):
    x = np.asarray(inputs["hidden_states"], dtype=np.float32)
    ws = {k: np.asarray(inputs[k], dtype=np.float32)
          for k in ("W_kv_d", "W_q_d", "W_k_u", "W_q_u", "W_v_u", "W_rope_k",
                    "W_rope_q", "W_o")}
    cosA, sinB = _rope_tables()
    masks = _mask_tiles()
    return [
        _prep_core_inputs(c, x, ws["W_kv_d"], ws["W_q_d"], ws["W_k_u"],
                          ws["W_q_u"], ws["W_v_u"], ws["W_rope_k"],
                          ws["W_rope_q"], ws["W_o"], cosA, sinB, masks)
        for c in range(N_CORES)
    ]


def assemble(results):
    """results: list of 8 dicts with 'out' [H, S] partials (transposed)."""
    full = np.empty((B, S, H), dtype=np.float32)
    for b in range(B):
        acc = results[4 * b]["out"].astype(np.float32)
        for g in range(1, 4):
            acc = acc + results[4 * b + g]["out"]
        full[b] = acc.T
    return full


def kernel(**inputs):
    nc = _get_nc()
    in_maps = make_in_maps(inputs)
    res = run_bass_kernel_spmd(nc, in_maps, core_ids=list(range(N_CORES)))
    return assemble(res.results)
